# revision 1
# baseline (speedup 1.0000x reference)
"""Trainium2 Bass kernel for multi-head attention (B=2, S=2048, D=1024, H=16, causal, RoPE).

Sharding: tensor-parallel over heads. Each of the 8 cores computes 2 heads
(128 of the 1024 q/k/v dims): QKV projections for its head slice, RoPE,
causal attention, and a partial output projection against its 128-column
slice of o_weight. The host sums the 8 partial outputs (the all-reduce).

Device-side layout choices:
  - Activations live transposed: qT/kT are [128 (head dims), B*S] so the
    scores matmul contracts dh on partitions. RoPE pairs are de-interleaved
    on the host (weight-row permutation) so pair partners sit 32 partitions
    apart; the rotate step is a single 128x128 sign-swap matmul (sperm).
  - Scores are computed transposed ([sk, sq]) so P = exp(scores) feeds the
    PV matmul directly as the moving operand; the two heads' K=64 score
    matmuls target row groups 0-63 / 64-127 and run concurrently on the PE
    array. V carries a block of 64 ones columns, so the PV matmul also
    emits the softmax denominator replicated across 64 partitions;
    normalization is then a plain reciprocal+multiply.
  - V is projected directly into [seq, dh] layout by using the (transposed)
    x tiles as the stationary operand - no on-chip transposes needed.
  - x and the q/k/v weights stream in as bf16; everything on-chip is
    float32r (FP22 multiply at full PE rate) with fp32 accumulation.
  - Work is software-pipelined: projection chunks run one chunk ahead of
    attention, output projections trail one chunk behind, both threaded
    through the attention tile loop so TensorE, ScalarE (exp), VectorE and
    GpSimd stay concurrently busy.
  - Partial outputs are written in bf16 (summed in fp32 on the host).
"""

import numpy as np

D_MODEL = 1024
N_HEADS = 16
D_HEAD = 64
THETA = 10000.0
B = 2
S = 2048
N_CORES = 8
BS = B * S  # 4096
NQ = 512    # query chunk width
NK = 128    # key tile width

_RT = {}


def _build():
    if _RT:
        return _RT
    import sys
    try:
        import concourse.bass  # noqa: F401
    except ImportError:
        sys.path.insert(0, "/opt/trn_rl_repo")
    import concourse.mybir as mybir
    import concourse.tile as tile
    from concourse import bacc
    from concourse._compat import axon_active
    from concourse.bass_utils import run_bass_kernel_spmd

    f32 = mybir.dt.float32
    f32r = mybir.dt.float32r
    bf16 = mybir.dt.bfloat16
    EXP = mybir.ActivationFunctionType.Exp

    nc = bacc.Bacc(
        "TRN2", target_bir_lowering=False, debug=not axon_active(),
        num_devices=N_CORES,
    )

    xT = nc.dram_tensor("xT", [D_MODEL, BS], bf16, kind="ExternalInput").ap()
    wq = nc.dram_tensor("wq", [D_MODEL, 128], bf16, kind="ExternalInput").ap()
    wk = nc.dram_tensor("wk", [D_MODEL, 128], bf16, kind="ExternalInput").ap()
    wv = nc.dram_tensor("wv", [D_MODEL, 128], bf16, kind="ExternalInput").ap()
    wo = nc.dram_tensor("wo", [128, D_MODEL], f32r, kind="ExternalInput").ap()
    trig = nc.dram_tensor("trig", [128, 2, S], f32, kind="ExternalInput").ap()
    sperm = nc.dram_tensor("sperm", [128, 128], f32r, kind="ExternalInput").ap()
    mask128 = nc.dram_tensor("mask128", [128, 128], f32, kind="ExternalInput").ap()
    y = nc.dram_tensor("y", [BS, D_MODEL], bf16, kind="ExternalOutput").ap()

    with tile.TileContext(nc) as tc:
        with (
            tc.tile_pool(name="singles", bufs=1) as singles,
            tc.tile_pool(name="px", bufs=3) as px,
            tc.tile_pool(name="ptmp", bufs=3) as ptmp,
            tc.tile_pool(name="pp", bufs=4) as pp,
            tc.tile_pool(name="pys", bufs=4) as pys,
            tc.tile_pool(name="pr", bufs=3) as pr,
            tc.tile_pool(name="ps_a", bufs=2, space="PSUM") as ps_a,
            tc.tile_pool(name="ps_s", bufs=2, space="PSUM") as ps_s,
            tc.tile_pool(name="ps_o", bufs=2, space="PSUM") as ps_o,
        ):
            wq_sb = singles.tile([128, 8, 128], bf16, tag="wq")
            wk_sb = singles.tile([128, 8, 128], bf16, tag="wk")
            wv_sb = singles.tile([128, 8, 128], bf16, tag="wv")
            wo_sb = singles.tile([128, D_MODEL], f32r, tag="wo")
            sperm_sb = singles.tile([128, 128], f32r, tag="sperm")
            m128_sb = singles.tile([128, 128], f32, tag="m128")
            qT_sb = singles.tile([128, BS], f32r, tag="qT")
            kT_sb = singles.tile([128, BS], f32r, tag="kT")
            oT_sb = singles.tile([128, BS], f32r, tag="oT")
            # V tiles: [seq-tile partitions, 32 tiles, 192]: cols 0:64 head A,
            # 64:128 ones, 128:192 head B. Head A lhsT = cols 0:128, head B
            # lhsT = cols 64:192; the ones block replicates the denominator.
            v_sb = singles.tile([128, 32, 192], f32r, tag="v")

            nc.scalar.dma_start(out=wq_sb, in_=wq.rearrange("(a p) m -> p a m", p=128))
            nc.scalar.dma_start(out=wk_sb, in_=wk.rearrange("(a p) m -> p a m", p=128))
            nc.scalar.dma_start(out=wv_sb, in_=wv.rearrange("(a p) m -> p a m", p=128))
            nc.scalar.dma_start(out=sperm_sb, in_=sperm)
            nc.scalar.dma_start(out=m128_sb, in_=mask128)
            nc.vector.memset(v_sb[:, :, 64:128].bitcast(f32), 1.0)

            def late_consts():
                nc.scalar.dma_start(out=wo_sb, in_=wo)

            def proj_pieces(b, c, after_xt=None):
                """QKV projections + rope + V transpose for seq chunk c of
                batch b (512 positions), as a list of closures that can be
                threaded through the attention tile loop."""
                u = 4 * b + c
                s0 = 512 * c
                csl = slice(512 * u, 512 * (u + 1))
                xt = []
                tmps = {}
                tg = [None]

                def p_load():
                    tg[0] = ptmp.tile([128, 2, 512], f32, tag="tg", name="tg")
                    nc.sync.dma_start(out=tg[0], in_=trig[:, :, s0:s0 + 512])
                    for h in range(2):
                        xth = px.tile([128, 4, 512], bf16, tag=f"xt{h}")
                        nc.sync.dma_start(
                            out=xth,
                            in_=xT[512 * h:512 * (h + 1), 512 * u:512 * (u + 1)]
                            .rearrange("(a p) n -> p a n", p=128),
                        )
                        xt.append(xth)
                    if after_xt is not None:
                        after_xt()

                def xslot(d):
                    return xt[d // 4][:, d % 4, :]

                def p_proj(w_sb, name):
                    def f():
                        ps = ps_a.tile([128, 512], f32, tag="pa")
                        for d in range(8):
                            nc.tensor.matmul(
                                ps, w_sb[:, d, :], xslot(d),
                                start=(d == 0), stop=(d == 7),
                            )
                        tmp = ptmp.tile([128, 512], f32r, tag=f"{name}tmp")
                        nc.scalar.copy(tmp, ps)  # PSUM -> SBUF on ACT
                        tmps[name] = tmp
                    return f

                def p_rope(name, dst):
                    def f():
                        tmp = tmps[name]
                        sq = ps_a.tile([128, 512], f32, tag="pa")
                        nc.tensor.matmul(sq, sperm_sb, tmp, start=True, stop=True)
                        cs = tg[0][:, 0, :]
                        sn = tg[0][:, 1, :]
                        m1 = ptmp.tile([128, 512], f32, tag="m1")
                        m2 = ptmp.tile([128, 512], f32, tag="m2")
                        nc.gpsimd.tensor_mul(m1, tmp, cs)
                        nc.vector.tensor_mul(m2, sq, sn)
                        nc.gpsimd.tensor_add(dst[:, csl], m1, m2)
                    return f

                vps = [None]

                def p_vproj(subs):
                    def f():
                        if vps[0] is None:
                            vps[0] = ps_a.tile([128, 4, 128], f32, tag="pa",
                                               name="vps")
                        for sub in subs:
                            for d in range(8):
                                nc.tensor.matmul(
                                    vps[0][:, sub, :],
                                    xslot(d)[:, 128 * sub:128 * (sub + 1)],
                                    wv_sb[:, d, :],
                                    start=(d == 0), stop=(d == 7),
                                )
                    return f

                def p_vstore_a():
                    nc.scalar.copy(v_sb[:, 4 * u:4 * u + 4, 0:64],
                                   vps[0][:, :, 0:64])

                def p_vstore_b():
                    nc.scalar.copy(v_sb[:, 4 * u:4 * u + 4, 128:192],
                                   vps[0][:, :, 64:128])

                return [p_load, p_proj(wq_sb, "q"), p_proj(wk_sb, "k"),
                        p_rope("q", qT_sb), p_vproj((0,)), p_vproj((1,)),
                        p_rope("k", kT_sb), p_vproj((2,)), p_vproj((3,)),
                        p_vstore_a, p_vstore_b]

            def proj_chunk(b, c, after_xt=None):
                for f in proj_pieces(b, c, after_xt):
                    f()

            def oproj_piece(b, c, s4):
                """Output projection for one 128-row seq tile (emitted one
                chunk late, spread across the next chunk's tiles)."""
                row0 = S * b + NQ * c + 128 * s4
                yp = ps_s.tile([128, 2, 512], f32, tag="sps")
                for hn in range(2):
                    nc.tensor.matmul(
                        yp[:, hn, :],
                        oT_sb[:, row0:row0 + 128],
                        wo_sb[:, 512 * hn:512 * (hn + 1)],
                        start=True, stop=True,
                    )
                ys = pys.tile([128, 1024], bf16, tag="ys")
                if s4 % 2 == 0:
                    nc.vector.tensor_copy(ys, yp.rearrange("p a n -> p (a n)"))
                else:
                    nc.scalar.copy(ys, yp.rearrange("p a n -> p (a n)"))
                nc.sync.dma_start(out=y[row0:row0 + 128, :], in_=ys)

            def attn_chunk(b, c, mids=(), fine_tail=False):
                """Causal attention for query chunk c of batch b. ``mids`` are
                emitted one per attention tile (pipelined filler work such as
                the previous chunk's output projection)."""
                mids = list(mids)
                qsl = slice(S * b + NQ * c, S * b + NQ * (c + 1))
                nt = (NQ // NK) * (c + 1)
                oa = ps_o.tile([128, 512], f32, tag="oacc")
                ob = ps_o.tile([128, 512], f32, tag="oacc")
                pending = []  # (p tile, j, t) awaiting PV matmul
                PV_DEPTH = 2

                def pv_flush():
                    p, j, _t = pending.pop(0)
                    w0 = 128 * j
                    nc.tensor.matmul(
                        oa[:, w0:512], v_sb[:, 16 * b + _t, 0:128],
                        p[:, 0, w0:512],
                        start=(_t == 0), stop=(_t == nt - 1),
                    )
                    nc.tensor.matmul(
                        ob[:, w0:512], v_sb[:, 16 * b + _t, 64:192],
                        p[:, 1, w0:512],
                        start=(_t == 0), stop=(_t == nt - 1),
                    )

                for t in range(nt):
                    ksl = slice(S * b + NK * t, S * b + NK * (t + 1))
                    j = max(0, t - 4 * c)  # within-chunk diagonal offset
                    w0 = 128 * j           # causally-dead query columns
                    qslj = slice(qsl.start + w0, qsl.stop)
                    sps = ps_s.tile([128, 2, 512], f32, tag="sps")
                    nc.tensor.matmul(
                        sps[:, 0, w0:512], kT_sb[0:64, ksl],
                        qT_sb[0:64, qslj], start=True, stop=True,
                    )
                    nc.tensor.matmul(
                        sps[:, 1, w0:512], kT_sb[64:128, ksl],
                        qT_sb[64:128, qslj], start=True, stop=True,
                    )
                    p = pp.tile([128, 2, 512], f32r, tag="p")
                    nc.scalar.activation(
                        p[:, :, w0:512], sps[:, :, w0:512], EXP, scale=0.125,
                    )
                    if t >= 4 * c:  # diagonal tile: mask boundary block
                        pb = p[:, :, w0:w0 + 128]
                        nc.vector.tensor_mul(
                            pb, pb, m128_sb.unsqueeze(1).to_broadcast([128, 2, 128]),
                        )
                    if len(pending) >= PV_DEPTH:
                        pv_flush()
                    pending.append((p, j, t))
                    if mids:
                        mids.pop(0)()
                while pending:
                    pv_flush()
                for m in mids:  # in case nt < len(mids)
                    m()

                # oa rows 64:128 / ob rows 0:64 hold the replicated
                # softmax denominators (from the ones block in V).
                rra = pr.tile([64, 512], f32, tag="rra")
                rrb = pr.tile([64, 512], f32, tag="rrb")
                if fine_tail:
                    # per-seq-tile normalize so the trailing output projection
                    # can start before the whole chunk is normalized
                    for s4 in range(4):
                        fs = slice(128 * s4, 128 * (s4 + 1))
                        qs4 = slice(qsl.start + 128 * s4, qsl.start + 128 * (s4 + 1))
                        nc.vector.reciprocal(rra[:, fs], oa[64:128, fs])
                        nc.vector.tensor_mul(oT_sb[0:64, qs4], oa[0:64, fs],
                                             rra[:, fs])
                        nc.vector.reciprocal(rrb[:, fs], ob[0:64, fs])
                        nc.vector.tensor_mul(oT_sb[64:128, qs4], ob[64:128, fs],
                                             rrb[:, fs])
                else:
                    nc.vector.reciprocal(rra, oa[64:128, :])
                    nc.vector.reciprocal(rrb, ob[0:64, :])
                    nc.vector.tensor_mul(oT_sb[0:64, qsl], oa[0:64, :], rra)
                    nc.vector.tensor_mul(oT_sb[64:128, qsl], ob[64:128, :], rrb)

            # Software pipeline: projections run one chunk ahead of attention;
            # output projections trail their attention chunk by one.
            def oproj_mids(bc):
                if bc is None:
                    return ()
                return [lambda s4=s4: oproj_piece(bc[0], bc[1], s4)
                        for s4 in range(4)]

            prev = None  # (b, c) whose oproj is still owed
            for b in range(B):
                if b == 0:
                    proj_chunk(b, 0, after_xt=late_consts)
                for c in range(4):
                    mids = list(oproj_mids(prev))
                    if c + 1 < 4:
                        pieces = proj_pieces(b, c + 1)
                    elif b + 1 < B:
                        # thread the next batch's first projection through
                        # this batch's last attention chunk
                        pieces = proj_pieces(b + 1, 0)
                    else:
                        pieces = []
                    merged = []
                    while pieces or mids:
                        if pieces:
                            merged.append(pieces.pop(0))
                        if mids:
                            merged.append(mids.pop(0))
                    mids = merged
                    attn_chunk(b, c, mids=mids,
                               fine_tail=True)
                    prev = (b, c)
            for s4 in range(4):
                oproj_piece(prev[0], prev[1], s4)

    nc.compile()
    _RT.update(
        nc=nc, run_bass_kernel_spmd=run_bass_kernel_spmd, mybir=mybir,
    )
    return _RT


def _host_inputs(q_weight, k_weight, v_weight, o_weight, in_features):
    """Build the per-core input maps (host-side sharding + layout prep)."""
    x = np.ascontiguousarray(np.asarray(in_features, dtype=np.float32))
    qw = np.asarray(q_weight, dtype=np.float32)
    kw = np.asarray(k_weight, dtype=np.float32)
    vw = np.asarray(v_weight, dtype=np.float32)
    ow = np.asarray(o_weight, dtype=np.float32)

    import ml_dtypes
    xT = np.ascontiguousarray(x.reshape(BS, D_MODEL).T).astype(ml_dtypes.bfloat16)

    perm64 = np.concatenate([np.arange(0, 64, 2), np.arange(1, 64, 2)])

    half = D_HEAD // 2
    inv_freq = THETA ** (-(np.arange(half, dtype=np.float64) * 2.0 / D_HEAD))
    pos = np.arange(S, dtype=np.float64)
    ang = pos[None, :] * inv_freq[:, None]        # [32, S]
    angf = np.tile(ang, (4, 1))                   # [128, S], row p -> i = p % 32
    trig = np.ascontiguousarray(np.stack(
        [np.cos(angf), np.sin(angf)], axis=1).astype(np.float32))

    spermT = np.zeros((128, 128), dtype=np.float32)
    for h in range(2):
        for i in range(32):
            spermT[h * 64 + 32 + i, h * 64 + i] = -1.0
            spermT[h * 64 + i, h * 64 + 32 + i] = 1.0

    kq = np.arange(128)
    mask128 = (np.arange(128)[None, :] >= kq[:, None]).astype(np.float32)

    shared = dict(xT=xT, trig=trig, sperm=spermT, mask128=mask128)

    in_maps = []
    for c in range(N_CORES):
        rows = slice(128 * c, 128 * (c + 1))

        def permqk(w):
            wc = w[rows]
            return np.ascontiguousarray(
                np.concatenate([wc[0:64][perm64], wc[64:128][perm64]]).T
            ).astype(ml_dtypes.bfloat16)

        in_maps.append(dict(
            shared,
            wq=permqk(qw),
            wk=permqk(kw),
            wv=np.ascontiguousarray(vw[rows].T).astype(ml_dtypes.bfloat16),
            wo=np.ascontiguousarray(ow[:, rows].T),
        ))
    return in_maps


def kernel(q_weight, k_weight, v_weight, o_weight, in_features):
    rt = _build()
    in_maps = _host_inputs(q_weight, k_weight, v_weight, o_weight, in_features)
    res = rt["run_bass_kernel_spmd"](
        rt["nc"], in_maps, core_ids=list(range(N_CORES)),
    )
    y = np.zeros((BS, D_MODEL), dtype=np.float32)
    for c in range(N_CORES):
        y += np.asarray(res.results[c]["y"], dtype=np.float32)
    return y.reshape(B, S, D_MODEL)



# revision 25
# speedup vs baseline: 1.1330x; 1.1330x over previous
"""Trainium2 Bass kernel for multi-head attention (B=2, S=2048, D=1024, H=16,
causal, RoPE).

Sharding: 2D (batch x head-quad). Core c handles batch b = c//4 and heads
4g..4g+3 where g = c%4 (256 of the 1024 q/k/v dims). Each core computes QKV
projections for its head slice, RoPE, causal attention, and a partial output
projection against its 256-column slice of o_weight. The host sums 4 partial
outputs per batch (the all-reduce). Versus 8-way head-only TP this halves the
x input, y output DMA and the output-copy work per core.

Device-side layout:
  - x for the core's batch is SBUF-resident: [128, 8, 2048] bf16 (slice d of
    the 1024 input dims on partitions, positions along free).
  - Heads are processed in pairs (128 dims): pair 0 = heads 4g,4g+1. qT/kT/oT
    are [128, 2*2048] bf16, columns 2048*pr + pos.
  - RoPE pairs are de-interleaved on the host (weight-row permutation) so pair
    partners sit 32 partitions apart; rotate = one 128x128 sign-swap matmul.
  - Scores are computed transposed ([sk, sq]); P = exp(scores) bf16 feeds PV
    directly. V carries a 64-wide ones block so the PV matmul also emits the
    softmax denominator; normalization = reciprocal + multiply on DVE.
  - Everything on-chip is bf16 except PSUM accumulators and reciprocals;
    this keeps all matmuls at 1 cycle/row and doubles DVE throughput where
    both operands are 16-bit.
  - Engine balance: ACT = exp + half the y-copy halves; DVE = rope m2, P
    masking, q/k PSUM drains, V stores, normalize, other y halves; Pool
    (no PSUM access) = rope m1/add; PE = matmuls only.
  - Software pipeline: steps (chunk c, pair pr); projections run one step
    ahead, output projections (needing both pairs) trail after odd steps,
    threaded through the attention tile loop.
"""

import numpy as np

D_MODEL = 1024
N_HEADS = 16
D_HEAD = 64
THETA = 10000.0
B = 2
S = 2048
N_CORES = 8
NQ = 512    # query chunk width
NK = 128    # key tile width

_RT = {}


def _build():
    if _RT:
        return _RT
    import sys
    try:
        import concourse.bass  # noqa: F401
    except ImportError:
        sys.path.insert(0, "/opt/trn_rl_repo")
    import concourse.mybir as mybir
    import concourse.tile as tile
    from concourse import bacc
    from concourse._compat import axon_active
    from concourse.bass_utils import run_bass_kernel_spmd

    f32 = mybir.dt.float32
    bf16 = mybir.dt.bfloat16
    fp8 = mybir.dt.float8e4
    EXP = mybir.ActivationFunctionType.Exp
    DR = mybir.MatmulPerfMode.DoubleRow

    nc = bacc.Bacc(
        "TRN2", target_bir_lowering=False, debug=not axon_active(),
        num_devices=N_CORES,
    )

    # x is shipped as an fp8 pair (x8, r8 = fp8(x - x8)); weights as w8
    # (duplicated for DoubleRow plane pairing) plus the residual wr in
    # adjacent-slice pairs. Projections compute w8*x8 + w8*r8 + wr*x8 via
    # fp8 DoubleRow matmuls (2 K-slices per pass); the dropped wr*r8 term
    # is ~0.07%. Weights are pre-scaled by 256 (fp8 denormal avoidance),
    # undone when PSUM is drained.
    xc = nc.dram_tensor("xc", [128, 2, 8, S], fp8, kind="ExternalInput").ap()
    wqx = nc.dram_tensor("wqx", [128, 8, 2, 256], fp8, kind="ExternalInput").ap()
    wqr = nc.dram_tensor("wqr", [128, 4, 2, 256], fp8, kind="ExternalInput").ap()
    wkx = nc.dram_tensor("wkx", [128, 8, 2, 256], fp8, kind="ExternalInput").ap()
    wkr = nc.dram_tensor("wkr", [128, 4, 2, 256], fp8, kind="ExternalInput").ap()
    wvx = nc.dram_tensor("wvx", [128, 8, 2, 256], fp8, kind="ExternalInput").ap()
    wvr = nc.dram_tensor("wvr", [128, 4, 2, 256], fp8, kind="ExternalInput").ap()
    wo = nc.dram_tensor("wo", [256, D_MODEL], bf16, kind="ExternalInput").ap()
    trig = nc.dram_tensor("trig", [128, 2, S], bf16, kind="ExternalInput").ap()
    sperm = nc.dram_tensor("sperm", [128, 128], bf16, kind="ExternalInput").ap()
    mask128 = nc.dram_tensor("mask128", [128, 128], bf16, kind="ExternalInput").ap()
    y = nc.dram_tensor("y", [S, D_MODEL], bf16, kind="ExternalOutput").ap()

    with tile.TileContext(nc) as tc:
        with (
            tc.tile_pool(name="singles", bufs=1) as singles,
            tc.tile_pool(name="ptmp", bufs=3) as ptmp,
            tc.tile_pool(name="pp", bufs=4) as pp,
            tc.tile_pool(name="pys", bufs=3) as pys,
            tc.tile_pool(name="prr", bufs=3) as prr,
            tc.tile_pool(name="ps_a", bufs=2, space="PSUM") as ps_a,
            tc.tile_pool(name="ps_s", bufs=2, space="PSUM") as ps_s,
            tc.tile_pool(name="ps_o", bufs=2, space="PSUM") as ps_o,
        ):
            xc_sb = singles.tile([128, 2, 8, S], fp8, tag="x")
            wqx_sb = singles.tile([128, 8, 2, 256], fp8, tag="wqx")
            wqr_sb = singles.tile([128, 4, 2, 256], fp8, tag="wqr")
            wkx_sb = singles.tile([128, 8, 2, 256], fp8, tag="wkx")
            wkr_sb = singles.tile([128, 4, 2, 256], fp8, tag="wkr")
            wvx_sb = singles.tile([128, 8, 2, 256], fp8, tag="wvx")
            wvr_sb = singles.tile([128, 4, 2, 256], fp8, tag="wvr")
            wo_sb = singles.tile([128, 2, D_MODEL], bf16, tag="wo")
            trig_sb = singles.tile([128, 2, S], bf16, tag="trig")
            sperm_sb = singles.tile([128, 128], bf16, tag="sperm")
            m128_sb = singles.tile([128, 128], bf16, tag="m128")
            qT_sb = singles.tile([128, 2 * S], bf16, tag="qT")
            kT_sb = singles.tile([128, 2 * S], bf16, tag="kT")
            oT_sb = singles.tile([128, 2 * S], bf16, tag="oT")
            # V tiles: [seq-tile partitions, 32 tiles, 192]: cols 0:64 head
            # even, 64:128 ones, 128:192 head odd. Head A lhsT = cols 0:128,
            # head B lhsT = cols 64:192; ones replicate the denominator.
            v_sb = singles.tile([128, 32, 192], bf16, tag="v")

            # Upfront DMAs, ordered so early compute unblocks first.
            def xsl(c):
                return slice(NQ * c, NQ * (c + 1))

            nc.sync.dma_start(out=wqx_sb, in_=wqx)
            nc.sync.dma_start(
                out=xc_sb[:, 0, 0:4, xsl(0)], in_=xc[:, 0, 0:4, xsl(0)])
            nc.sync.dma_start(
                out=xc_sb[:, 1, 0:4, xsl(0)], in_=xc[:, 1, 0:4, xsl(0)])
            nc.sync.dma_start(
                out=xc_sb[:, 0, 4:8, xsl(0)], in_=xc[:, 0, 4:8, xsl(0)])
            nc.sync.dma_start(
                out=xc_sb[:, 1, 4:8, xsl(0)], in_=xc[:, 1, 4:8, xsl(0)])
            nc.sync.dma_start(out=wqr_sb, in_=wqr)
            nc.sync.dma_start(out=wkx_sb, in_=wkx)
            nc.sync.dma_start(out=sperm_sb, in_=sperm)
            nc.sync.dma_start(
                out=trig_sb[:, :, 0:1024], in_=trig[:, :, 0:1024])
            nc.sync.dma_start(out=wkr_sb, in_=wkr)
            nc.sync.dma_start(out=wvx_sb, in_=wvx)
            nc.sync.dma_start(out=wvr_sb, in_=wvr)
            nc.sync.dma_start(out=m128_sb, in_=mask128)
            nc.sync.dma_start(
                out=trig_sb[:, :, 1024:2048], in_=trig[:, :, 1024:2048])
            nc.sync.dma_start(out=wo_sb, in_=wo.rearrange("(q p) j -> p q j", p=128))
            for c in range(1, 4):
                for j in range(2):
                    nc.sync.dma_start(
                        out=xc_sb[:, j, :, xsl(c)], in_=xc[:, j, :, xsl(c)])
            nc.gpsimd.memset(v_sb[:, :, 64:128], 1.0)

            def proj_pieces(pr, c):
                """QKV projections + rope + V for (pair pr, seq chunk c), as
                closures threaded through the attention tile loop."""
                csl = xsl(c)
                dsl = slice(S * pr + NQ * c, S * pr + NQ * (c + 1))
                wsl = slice(128 * pr, 128 * (pr + 1))
                tmps = {}

                def p_proj(wx_sb, wr_sb, name):
                    def f():
                        ps = ps_a.tile([128, 512], f32, tag="pa")
                        for d in range(8):
                            nc.tensor.matmul(
                                ps, wx_sb[:, d, :, wsl], xc_sb[:, :, d, csl],
                                start=(d == 0), stop=False, perf_mode=DR,
                            )
                        for e in range(4):
                            nc.tensor.matmul(
                                ps, wr_sb[:, e, :, wsl],
                                xc_sb[:, 0, 2 * e:2 * e + 2, csl],
                                start=False, stop=(e == 3), perf_mode=DR,
                            )
                        tmp = ptmp.tile([128, 512], bf16, tag=f"{name}tmp")
                        nc.vector.tensor_scalar_mul(tmp, ps, 1.0 / 256.0)
                        tmps[name] = tmp
                    return f

                def p_rope(name, dst):
                    def f():
                        tmp = tmps[name]
                        sq = ps_a.tile([128, 512], f32, tag="pa")
                        nc.tensor.matmul(sq, sperm_sb, tmp, start=True, stop=True)
                        cs = trig_sb[:, 0, csl]
                        sn = trig_sb[:, 1, csl]
                        m1 = ptmp.tile([128, 512], bf16, tag="m1")
                        m2 = ptmp.tile([128, 512], bf16, tag="m2")
                        nc.gpsimd.tensor_mul(m1, tmp, cs)
                        nc.vector.tensor_mul(m2, sq, sn)
                        nc.vector.tensor_add(dst[:, dsl], m1, m2)
                    return f

                def p_vproj(sub):
                    def f():
                        vp = ps_a.tile([128, 128], f32, tag="pa")
                        ssl = slice(NQ * c + 128 * sub, NQ * c + 128 * (sub + 1))
                        for d in range(8):
                            nc.tensor.matmul(
                                vp, xc_sb[:, :, d, ssl], wvx_sb[:, d, :, wsl],
                                start=(d == 0), stop=False, perf_mode=DR,
                            )
                        for e in range(4):
                            nc.tensor.matmul(
                                vp, xc_sb[:, 0, 2 * e:2 * e + 2, ssl],
                                wvr_sb[:, e, :, wsl],
                                start=False, stop=(e == 3), perf_mode=DR,
                            )
                        ti = 16 * pr + 4 * c + sub
                        nc.vector.tensor_scalar_mul(
                            v_sb[:, ti, 0:64], vp[:, 0:64], 1.0 / 256.0)
                        nc.vector.tensor_scalar_mul(
                            v_sb[:, ti, 128:192], vp[:, 64:128], 1.0 / 256.0)
                    return f

                return [p_proj(wqx_sb, wqr_sb, "q"), p_proj(wkx_sb, wkr_sb, "k"),
                        p_rope("q", qT_sb), p_vproj(0), p_vproj(1),
                        p_rope("k", kT_sb), p_vproj(2), p_vproj(3)]

            def oproj_piece(c, s4, tail=False):
                """Output projection for one 128-row seq tile (contracts both
                pairs' oT slices; emitted after both pairs finish chunk c).
                The two 512-col halves drain PSUM on ACT and DVE in parallel
                (both on ACT in the tail, where exp is done but DVE is busy
                normalizing)."""
                row0 = NQ * c + 128 * s4
                ys = pys.tile([128, D_MODEL], bf16, tag="ys")
                for hn in range(2):
                    yph = ps_a.tile([128, 512], f32, tag="pa")
                    for grp in range(2):
                        nc.tensor.matmul(
                            yph,
                            oT_sb[:, S * grp + row0:S * grp + row0 + 128],
                            wo_sb[:, grp, 512 * hn:512 * (hn + 1)],
                            start=(grp == 0), stop=(grp == 1),
                        )
                    hsl = slice(512 * hn, 512 * (hn + 1))
                    if tail or (hn + s4) % 2 == 0:
                        nc.scalar.copy(ys[:, hsl], yph)
                    else:
                        nc.vector.tensor_copy(ys[:, hsl], yph)
                nc.sync.dma_start(out=y[row0:row0 + 128, :], in_=ys)

            def attn_chunk(pr, c, mids=(), fine_tail=None):
                """Causal attention for (pair pr, query chunk c). ``mids`` are
                emitted one per attention tile (pipelined filler work).
                ``fine_tail``: optional list of closures interleaved with
                per-s4 normalization at the end (for the final chunk)."""
                mids = list(mids)
                qoff = S * pr + NQ * c
                koff = S * pr
                nt = (NQ // NK) * (c + 1)
                oa = ps_o.tile([128, 512], f32, tag="oacc")
                ob = ps_o.tile([128, 512], f32, tag="oacc")
                pending = []  # (p tile, j, t) awaiting PV matmul
                PV_DEPTH = 3

                def pv_flush():
                    p, j, _t = pending.pop(0)
                    w0 = 128 * j
                    nc.tensor.matmul(
                        oa[:, w0:512], v_sb[:, 16 * pr + _t, 0:128],
                        p[:, 0, w0:512],
                        start=(_t == 0), stop=(_t == nt - 1),
                    )
                    nc.tensor.matmul(
                        ob[:, w0:512], v_sb[:, 16 * pr + _t, 64:192],
                        p[:, 1, w0:512],
                        start=(_t == 0), stop=(_t == nt - 1),
                    )

                for t in range(nt):
                    ksl = slice(koff + NK * t, koff + NK * (t + 1))
                    j = max(0, t - 4 * c)  # within-chunk diagonal offset
                    w0 = 128 * j           # causally-dead query columns
                    qslj = slice(qoff + w0, qoff + 512)
                    sps = ps_s.tile([128, 2, 512], f32, tag="sps")
                    nc.tensor.matmul(
                        sps[:, 0, w0:512], kT_sb[0:64, ksl],
                        qT_sb[0:64, qslj], start=True, stop=True,
                    )
                    nc.tensor.matmul(
                        sps[:, 1, w0:512], kT_sb[64:128, ksl],
                        qT_sb[64:128, qslj], start=True, stop=True,
                    )
                    p = pp.tile([128, 2, 512], bf16, tag="p")
                    nc.scalar.activation(
                        p[:, :, w0:512], sps[:, :, w0:512], EXP, scale=0.125,
                    )
                    if t >= 4 * c:  # diagonal tile: mask boundary block
                        pb = p[:, :, w0:w0 + 128]
                        nc.vector.tensor_mul(
                            pb, pb, m128_sb.unsqueeze(1).to_broadcast([128, 2, 128]),
                        )
                    if len(pending) >= PV_DEPTH:
                        pv_flush()
                    pending.append((p, j, t))
                    if mids:
                        mids.pop(0)()
                while pending:
                    pv_flush()
                for m in mids:  # in case nt < len(mids)
                    m()

                # oa rows 64:128 / ob rows 0:64 hold the replicated softmax
                # denominators (from the ones block in V).
                if fine_tail is None:
                    qsl = slice(qoff, qoff + 512)
                    rra = prr.tile([64, 512], f32, tag="rra")
                    rrb = prr.tile([64, 512], f32, tag="rrb")
                    nc.vector.reciprocal(rra, oa[64:128, :])
                    nc.vector.tensor_mul(oT_sb[0:64, qsl], oa[0:64, :], rra)
                    nc.vector.reciprocal(rrb, ob[0:64, :])
                    nc.vector.tensor_mul(oT_sb[64:128, qsl], ob[64:128, :], rrb)
                else:
                    # per-seq-tile normalize so trailing work (the final
                    # output projections) can start before the whole chunk
                    # is normalized
                    for s4 in range(4):
                        fs = slice(128 * s4, 128 * (s4 + 1))
                        qs4 = slice(qoff + 128 * s4, qoff + 128 * (s4 + 1))
                        rra = prr.tile([64, 128], f32, tag="rra")
                        rrb = prr.tile([64, 128], f32, tag="rrb")
                        nc.vector.reciprocal(rra, oa[64:128, fs])
                        nc.vector.tensor_mul(oT_sb[0:64, qs4], oa[0:64, fs], rra)
                        nc.vector.reciprocal(rrb, ob[0:64, fs])
                        nc.vector.tensor_mul(oT_sb[64:128, qs4], ob[64:128, fs],
                                             rrb)
                        fine_tail[s4](tail=True)

            # Software pipeline over steps (c, pr): projections one step
            # ahead; output projections for chunk c trail after step (c, 1),
            # threaded through step (c+1, 1) (keeps ACT exp the pacer on the
            # lighter even steps and fills the mid-less last step).
            steps = [(c, pr) for c in range(4) for pr in range(2)]
            for s, (c, pr) in enumerate(steps):
                if s == 0:
                    for f in proj_pieces(0, 0):
                        f()
                mids = []
                if s + 1 < len(steps):
                    c2, pr2 = steps[s + 1]
                    mids = proj_pieces(pr2, c2)
                if pr == 1 and c >= 1:
                    opieces = [lambda s4=s4, cc=c - 1: oproj_piece(cc, s4)
                               for s4 in range(4)]
                    merged = []
                    while mids or opieces:
                        if mids:
                            merged.append(mids.pop(0))
                        if opieces:
                            merged.append(opieces.pop(0))
                    mids = merged
                fine = None
                if s == len(steps) - 1:
                    fine = [lambda tail=False, s4=s4: oproj_piece(3, s4, tail)
                            for s4 in range(4)]
                attn_chunk(pr, c, mids, fine_tail=fine)

    nc.compile()
    _RT.update(
        nc=nc, run_bass_kernel_spmd=run_bass_kernel_spmd, mybir=mybir,
    )
    return _RT


def _host_inputs(q_weight, k_weight, v_weight, o_weight, in_features):
    """Build the per-core input maps (host-side sharding + layout prep)."""
    import ml_dtypes
    bf = ml_dtypes.bfloat16
    x = np.ascontiguousarray(np.asarray(in_features, dtype=np.float32))
    qw = np.asarray(q_weight, dtype=np.float32)
    kw = np.asarray(k_weight, dtype=np.float32)
    vw = np.asarray(v_weight, dtype=np.float32)
    ow = np.asarray(o_weight, dtype=np.float32)

    perm64 = np.concatenate([np.arange(0, 64, 2), np.arange(1, 64, 2)])

    half = D_HEAD // 2
    inv_freq = THETA ** (-(np.arange(half, dtype=np.float64) * 2.0 / D_HEAD))
    pos = np.arange(S, dtype=np.float64)
    ang = inv_freq[:, None] * pos[None, :]        # [32, S]
    angf = np.tile(ang, (4, 1))                   # [128, S], row p -> i = p % 32
    trig = np.ascontiguousarray(np.stack(
        [np.cos(angf), np.sin(angf)], axis=1).astype(bf))

    spermT = np.zeros((128, 128), dtype=np.float32)
    for h in range(2):
        for i in range(32):
            spermT[h * 64 + 32 + i, h * 64 + i] = -1.0
            spermT[h * 64 + i, h * 64 + 32 + i] = 1.0
    spermT = spermT.astype(bf)

    kq = np.arange(128)
    mask128 = (np.arange(128)[None, :] >= kq[:, None]).astype(bf)

    f8 = ml_dtypes.float8_e4m3fn

    def xcomp(xb):
        """x as an fp8 (value, residual) pair in [128, 2, 8, S] layout."""
        xt = np.ascontiguousarray(xb.reshape(S, D_MODEL).T)  # [1024, S]
        x8 = xt.astype(f8)
        r8 = (xt - x8.astype(np.float32)).astype(f8)
        comp = np.stack([x8.reshape(8, 128, S), r8.reshape(8, 128, S)], axis=0)
        return np.ascontiguousarray(comp.transpose(2, 0, 1, 3))  # [128,2,8,S]

    def wcomp(wl):
        """256*w as fp8 w8 (duplicated DoubleRow planes) + residual pairs."""
        w256 = 256.0 * wl  # [1024, 256]
        w8 = w256.astype(f8)
        wr = (w256 - w8.astype(np.float32)).astype(f8)
        w8p = w8.reshape(8, 128, 256).transpose(1, 0, 2)      # [128, 8, 256]
        wx = np.ascontiguousarray(np.stack([w8p, w8p], axis=2))  # [128,8,2,256]
        wrp = wr.reshape(4, 2, 128, 256).transpose(2, 0, 1, 3)
        return wx, np.ascontiguousarray(wrp)                   # [128,4,2,256]

    xcb = [xcomp(x[b]) for b in range(B)]

    in_maps = []
    for core in range(N_CORES):
        b, g = divmod(core, 4)
        rows = slice(256 * g, 256 * (g + 1))

        def permqk(w):
            wc = w[rows]
            blocks = [wc[64 * h:64 * (h + 1)][perm64] for h in range(4)]
            return np.ascontiguousarray(np.concatenate(blocks).T)

        wqx, wqr = wcomp(permqk(qw))
        wkx, wkr = wcomp(permqk(kw))
        wvx, wvr = wcomp(np.ascontiguousarray(vw[rows].T))

        in_maps.append(dict(
            xc=xcb[b],
            wqx=wqx, wqr=wqr, wkx=wkx, wkr=wkr, wvx=wvx, wvr=wvr,
            wo=np.ascontiguousarray(ow[:, rows].T).astype(bf),
            trig=trig, sperm=spermT, mask128=mask128,
        ))
    return in_maps


def kernel(q_weight, k_weight, v_weight, o_weight, in_features):
    rt = _build()
    in_maps = _host_inputs(q_weight, k_weight, v_weight, o_weight, in_features)
    res = rt["run_bass_kernel_spmd"](
        rt["nc"], in_maps, core_ids=list(range(N_CORES)),
    )
    y = np.zeros((B, S, D_MODEL), dtype=np.float32)
    for core in range(N_CORES):
        y[core // 4] += np.asarray(res.results[core]["y"], dtype=np.float32)
    return y


# revision 38
# speedup vs baseline: 1.2164x; 1.0736x over previous
"""Trainium2 Bass kernel for multi-head attention (B=2, S=2048, D=1024, H=16,
causal, RoPE).

Sharding: 2D (batch x head-quad). Core c handles batch b = c//4 and heads
4g..4g+3 where g = c%4 (256 of the 1024 q/k/v dims). Each core computes QKV
projections for its head slice, RoPE, causal attention, and a partial output
projection against its 256-column slice of o_weight. The host sums 4 partial
outputs per batch (the all-reduce). Versus 8-way head-only TP this halves the
x input, y output DMA and the output-copy work per core.

Device-side layout:
  - x for the core's batch is SBUF-resident: [128, 8, 2048] bf16 (slice d of
    the 1024 input dims on partitions, positions along free).
  - Heads are processed in pairs (128 dims): pair 0 = heads 4g,4g+1. qT/kT/oT
    are [128, 2*2048] bf16, columns 2048*pr + pos.
  - RoPE pairs are de-interleaved on the host (weight-row permutation) so pair
    partners sit 32 partitions apart; rotate = one 128x128 sign-swap matmul.
  - Scores are computed transposed ([sk, sq]); P = exp(scores) bf16 feeds PV
    directly. V carries a 64-wide ones block so the PV matmul also emits the
    softmax denominator; normalization = reciprocal + multiply on DVE.
  - Everything on-chip is bf16 except PSUM accumulators and reciprocals;
    this keeps all matmuls at 1 cycle/row and doubles DVE throughput where
    both operands are 16-bit.
  - Engine balance: ACT = exp + half the y-copy halves; DVE = rope m2, P
    masking, q/k PSUM drains, V stores, normalize, other y halves; Pool
    (no PSUM access) = rope m1/add; PE = matmuls only.
  - Software pipeline: steps (chunk c, pair pr); projections run one step
    ahead, output projections (needing both pairs) trail after odd steps,
    threaded through the attention tile loop.
"""

import numpy as np

D_MODEL = 1024
N_HEADS = 16
D_HEAD = 64
THETA = 10000.0
B = 2
S = 2048
N_CORES = 8
NQ = 512    # query chunk width
NK = 128    # key tile width

_RT = {}
MIDSPREAD = 0
PVD = 3
USE_ACT_EARLY = True
MM_LABELS = []  # debug: label per emitted matmul, in program order


def _build():
    if _RT:
        return _RT
    import sys
    try:
        import concourse.bass  # noqa: F401
    except ImportError:
        sys.path.insert(0, "/opt/trn_rl_repo")
    import concourse.mybir as mybir
    import concourse.tile as tile
    from concourse import bacc
    from concourse._compat import axon_active
    from concourse.bass_utils import run_bass_kernel_spmd

    f32 = mybir.dt.float32
    bf16 = mybir.dt.bfloat16
    fp8 = mybir.dt.float8e4
    EXP = mybir.ActivationFunctionType.Exp
    DR = mybir.MatmulPerfMode.DoubleRow

    nc = bacc.Bacc(
        "TRN2", target_bir_lowering=False, debug=not axon_active(),
        num_devices=N_CORES,
    )

    # x is shipped as an fp8 pair (x8, r8 = fp8(x - x8)); weights as w8
    # (duplicated for DoubleRow plane pairing) plus the residual wr in
    # adjacent-slice pairs. Projections compute w8*x8 + w8*r8 + wr*x8 via
    # fp8 DoubleRow matmuls (2 K-slices per pass); the dropped wr*r8 term
    # is ~0.07%. Weights are pre-scaled by 256 (fp8 denormal avoidance),
    # undone when PSUM is drained.
    xc = nc.dram_tensor("xc", [128, 2, 8, S], fp8, kind="ExternalInput").ap()
    wqx = nc.dram_tensor("wqx", [128, 8, 2, 256], fp8, kind="ExternalInput").ap()
    wqr = nc.dram_tensor("wqr", [128, 4, 2, 256], fp8, kind="ExternalInput").ap()
    wkx = nc.dram_tensor("wkx", [128, 8, 2, 256], fp8, kind="ExternalInput").ap()
    wkr = nc.dram_tensor("wkr", [128, 4, 2, 256], fp8, kind="ExternalInput").ap()
    wvx = nc.dram_tensor("wvx", [128, 8, 2, 256], fp8, kind="ExternalInput").ap()
    wvr = nc.dram_tensor("wvr", [128, 4, 2, 256], fp8, kind="ExternalInput").ap()
    wo = nc.dram_tensor("wo", [256, D_MODEL], bf16, kind="ExternalInput").ap()
    trig = nc.dram_tensor("trig", [128, 2, S], bf16, kind="ExternalInput").ap()
    sperm = nc.dram_tensor("sperm", [128, 128], bf16, kind="ExternalInput").ap()
    mask128 = nc.dram_tensor("mask128", [128, 128], bf16, kind="ExternalInput").ap()
    y = nc.dram_tensor("y", [S, D_MODEL], bf16, kind="ExternalOutput").ap()

    def MM(label, *args, **kw):
        MM_LABELS.append(label)
        return nc.tensor.matmul(*args, **kw)

    with tile.TileContext(nc) as tc:
        with (
            tc.tile_pool(name="singles", bufs=1) as singles,
            tc.tile_pool(name="ptmp", bufs=3) as ptmp,
            tc.tile_pool(name="pp", bufs=6) as pp,
            tc.tile_pool(name="pys", bufs=3) as pys,
            tc.tile_pool(name="prr", bufs=3) as prr,
            tc.tile_pool(name="ps_a", bufs=2, space="PSUM") as ps_a,
            tc.tile_pool(name="ps_s", bufs=2, space="PSUM") as ps_s,
            tc.tile_pool(name="ps_o", bufs=2, space="PSUM") as ps_o,
        ):
            xc_sb = singles.tile([128, 2, 8, S], fp8, tag="x")
            wqx_sb = singles.tile([128, 8, 2, 256], fp8, tag="wqx")
            wqr_sb = singles.tile([128, 4, 2, 256], fp8, tag="wqr")
            wkx_sb = singles.tile([128, 8, 2, 256], fp8, tag="wkx")
            wkr_sb = singles.tile([128, 4, 2, 256], fp8, tag="wkr")
            wvx_sb = singles.tile([128, 8, 2, 256], fp8, tag="wvx")
            wvr_sb = singles.tile([128, 4, 2, 256], fp8, tag="wvr")
            wo_sb = singles.tile([128, 2, D_MODEL], bf16, tag="wo")
            trig_sb = singles.tile([128, 2, S], bf16, tag="trig")
            sperm_sb = singles.tile([128, 128], bf16, tag="sperm")
            m128_sb = singles.tile([128, 128], bf16, tag="m128")
            qT_sb = singles.tile([128, 2 * S], bf16, tag="qT")
            kT_sb = singles.tile([128, 2 * S], bf16, tag="kT")
            oT_sb = singles.tile([128, 2 * S], bf16, tag="oT")
            # V tiles: [seq-tile partitions, 32 tiles, 192]: cols 0:64 head
            # even, 64:128 ones, 128:192 head odd. Head A lhsT = cols 0:128,
            # head B lhsT = cols 64:192; ones replicate the denominator.
            v_sb = singles.tile([128, 32, 192], bf16, tag="v")

            # Warm the ACT exp table at t=0 (the first real exp would
            # otherwise pay the ~1.3us table load mid-pipeline).
            warm = singles.tile([1, 16], f32, tag="warm")
            nc.vector.memset(warm, 0.0)
            nc.scalar.activation(warm, warm, EXP)

            # Upfront DMAs, ordered so early compute unblocks first.
            def xsl(c):
                return slice(NQ * c, NQ * (c + 1))

            nc.sync.dma_start(out=wqx_sb, in_=wqx)
            nc.sync.dma_start(
                out=xc_sb[:, 0, 0:4, xsl(0)], in_=xc[:, 0, 0:4, xsl(0)])
            nc.sync.dma_start(
                out=xc_sb[:, 1, 0:4, xsl(0)], in_=xc[:, 1, 0:4, xsl(0)])
            nc.sync.dma_start(
                out=xc_sb[:, 0, 4:8, xsl(0)], in_=xc[:, 0, 4:8, xsl(0)])
            nc.sync.dma_start(
                out=xc_sb[:, 1, 4:8, xsl(0)], in_=xc[:, 1, 4:8, xsl(0)])
            nc.sync.dma_start(out=wqr_sb, in_=wqr)
            nc.sync.dma_start(out=wkx_sb, in_=wkx)
            nc.sync.dma_start(out=sperm_sb, in_=sperm)
            nc.sync.dma_start(
                out=trig_sb[:, :, 0:1024], in_=trig[:, :, 0:1024])
            nc.sync.dma_start(out=wkr_sb, in_=wkr)
            nc.sync.dma_start(out=wvx_sb, in_=wvx)
            nc.sync.dma_start(out=wvr_sb, in_=wvr)
            nc.sync.dma_start(out=m128_sb, in_=mask128)
            nc.sync.dma_start(
                out=trig_sb[:, :, 1024:2048], in_=trig[:, :, 1024:2048])
            nc.sync.dma_start(out=wo_sb, in_=wo.rearrange("(q p) j -> p q j", p=128))
            for c in range(1, 4):
                for j in range(2):
                    nc.sync.dma_start(
                        out=xc_sb[:, j, :, xsl(c)], in_=xc[:, j, :, xsl(c)])
            nc.gpsimd.memset(v_sb[:, :, 64:128], 1.0)

            def proj_pieces(pr, c):
                """QKV projections + rope + V for (pair pr, seq chunk c), as
                closures threaded through the attention tile loop."""
                csl = xsl(c)
                dsl = slice(S * pr + NQ * c, S * pr + NQ * (c + 1))
                wsl = slice(128 * pr, 128 * (pr + 1))
                tmps = {}

                def p_proj(wx_sb, wr_sb, name):
                    def f():
                        ps = ps_a.tile([128, 512], f32, tag="pa")
                        for d in range(8):
                            nc.tensor.matmul(
                                ps, wx_sb[:, d, :, wsl], xc_sb[:, :, d, csl],
                                start=(d == 0), stop=False, perf_mode=DR,
                            )
                        for e in range(4):
                            nc.tensor.matmul(
                                ps, wr_sb[:, e, :, wsl],
                                xc_sb[:, 0, 2 * e:2 * e + 2, csl],
                                start=False, stop=(e == 3), perf_mode=DR,
                            )
                        tmp = ptmp.tile([128, 512], bf16, tag=f"{name}tmp")
                        nc.vector.tensor_scalar_mul(tmp, ps, 1.0 / 256.0)
                        tmps[name] = tmp
                    return f

                def p_rope(name, dst):
                    def f():
                        tmp = tmps[name]
                        sq = ps_a.tile([128, 512], f32, tag="pa")
                        nc.tensor.matmul(sq, sperm_sb, tmp, start=True, stop=True)
                        cs = trig_sb[:, 0, csl]
                        sn = trig_sb[:, 1, csl]
                        m1 = ptmp.tile([128, 512], bf16, tag="m1")
                        m2 = ptmp.tile([128, 512], bf16, tag="m2")
                        nc.gpsimd.tensor_mul(m1, tmp, cs)
                        nc.vector.tensor_mul(m2, sq, sn)
                        nc.vector.tensor_add(dst[:, dsl], m1, m2)
                    return f

                def p_vproj(sub):
                    def f():
                        vp = ps_a.tile([128, 128], f32, tag="pa")
                        ssl = slice(NQ * c + 128 * sub, NQ * c + 128 * (sub + 1))
                        for d in range(8):
                            nc.tensor.matmul(
                                vp, xc_sb[:, :, d, ssl], wvx_sb[:, d, :, wsl],
                                start=(d == 0), stop=False, perf_mode=DR,
                            )
                        for e in range(4):
                            nc.tensor.matmul(
                                vp, xc_sb[:, 0, 2 * e:2 * e + 2, ssl],
                                wvr_sb[:, e, :, wsl],
                                start=False, stop=(e == 3), perf_mode=DR,
                            )
                        ti = 16 * pr + 4 * c + sub
                        nc.vector.tensor_scalar_mul(
                            v_sb[:, ti, 0:64], vp[:, 0:64], 1.0 / 256.0)
                        nc.vector.tensor_scalar_mul(
                            v_sb[:, ti, 128:192], vp[:, 64:128], 1.0 / 256.0)
                    return f

                return [p_proj(wqx_sb, wqr_sb, "q"), p_proj(wkx_sb, wkr_sb, "k"),
                        p_rope("q", qT_sb), p_vproj(0), p_vproj(1),
                        p_rope("k", kT_sb), p_vproj(2), p_vproj(3)]

            def oproj_piece(c, s4, tail=False):
                """Output projection for one 128-row seq tile (contracts both
                pairs' oT slices; emitted after both pairs finish chunk c).
                The two 512-col halves drain PSUM on ACT and DVE in parallel
                (both on ACT in the tail, where exp is done but DVE is busy
                normalizing)."""
                row0 = NQ * c + 128 * s4
                ys = pys.tile([128, D_MODEL], bf16, tag="ys")
                for hn in range(2):
                    yph = ps_a.tile([128, 512], f32, tag="pa")
                    for grp in range(2):
                        nc.tensor.matmul(
                            yph,
                            oT_sb[:, S * grp + row0:S * grp + row0 + 128],
                            wo_sb[:, grp, 512 * hn:512 * (hn + 1)],
                            start=(grp == 0), stop=(grp == 1),
                        )
                    hsl = slice(512 * hn, 512 * (hn + 1))
                    if tail or (hn + s4) % 2 == 0:
                        nc.scalar.copy(ys[:, hsl], yph)
                    else:
                        nc.vector.tensor_copy(ys[:, hsl], yph)
                nc.sync.dma_start(out=y[row0:row0 + 128, :], in_=ys)

            def attn_chunk(pr, c, mids=(), fine_tail=None):
                """Causal attention for (pair pr, query chunk c). ``mids`` are
                emitted one per attention tile (pipelined filler work).
                ``fine_tail``: optional list of closures interleaved with
                per-s4 normalization at the end (for the final chunk)."""
                mids = list(mids)
                qoff = S * pr + NQ * c
                koff = S * pr
                nt = (NQ // NK) * (c + 1)
                oa = ps_o.tile([128, 512], f32, tag="oacc")
                ob = ps_o.tile([128, 512], f32, tag="oacc")
                pending = []  # (p tile, j, t) awaiting PV matmul
                PV_DEPTH = PVD if c else PVD0

                def pv_flush():
                    p, j, _t = pending.pop(0)
                    w0 = 128 * j
                    nc.tensor.matmul(
                        oa[:, w0:512], v_sb[:, 16 * pr + _t, 0:128],
                        p[:, 0, w0:512],
                        start=(_t == 0), stop=(_t == nt - 1),
                    )
                    nc.tensor.matmul(
                        ob[:, w0:512], v_sb[:, 16 * pr + _t, 64:192],
                        p[:, 1, w0:512],
                        start=(_t == 0), stop=(_t == nt - 1),
                    )

                for t in range(nt):
                    ksl = slice(koff + NK * t, koff + NK * (t + 1))
                    j = max(0, t - 4 * c)  # within-chunk diagonal offset
                    w0 = 128 * j           # causally-dead query columns
                    qslj = slice(qoff + w0, qoff + 512)
                    sps = ps_s.tile([128, 2, 512], f32, tag="sps")
                    nc.tensor.matmul(
                        sps[:, 0, w0:512], kT_sb[0:64, ksl],
                        qT_sb[0:64, qslj], start=True, stop=True,
                    )
                    nc.tensor.matmul(
                        sps[:, 1, w0:512], kT_sb[64:128, ksl],
                        qT_sb[64:128, qslj], start=True, stop=True,
                    )
                    p = pp.tile([128, 2, 512], bf16, tag="p")
                    nc.scalar.activation(
                        p[:, :, w0:512], sps[:, :, w0:512], EXP, scale=0.125,
                    )
                    if t >= 4 * c:  # diagonal tile: mask boundary block
                        pb = p[:, :, w0:w0 + 128]
                        nc.vector.tensor_mul(
                            pb, pb, m128_sb.unsqueeze(1).to_broadcast([128, 2, 128]),
                        )
                    depth = PV_DEPTH
                    if fine_tail is not None and t >= nt - TAIL_DRAIN:
                        depth = 1
                    while len(pending) >= depth:
                        pv_flush()
                    pending.append((p, j, t))
                    if mids:
                        mids.pop(0)()
                while pending:
                    pv_flush()
                for m in mids:  # in case nt < len(mids)
                    m()

                # oa rows 64:128 / ob rows 0:64 hold the replicated softmax
                # denominators (from the ones block in V).
                if fine_tail is None:
                    qsl = slice(qoff, qoff + 512)
                    rra = prr.tile([64, 512], f32, tag="rra")
                    rrb = prr.tile([64, 512], f32, tag="rrb")
                    nc.vector.reciprocal(rra, oa[64:128, :])
                    nc.vector.tensor_mul(oT_sb[0:64, qsl], oa[0:64, :], rra)
                    nc.vector.reciprocal(rrb, ob[0:64, :])
                    nc.vector.tensor_mul(oT_sb[64:128, qsl], ob[64:128, :], rrb)
                else:
                    # per-seq-tile normalize so trailing work (the final
                    # output projections) can start before the whole chunk
                    # is normalized
                    for s4 in range(4):
                        fs = slice(128 * s4, 128 * (s4 + 1))
                        qs4 = slice(qoff + 128 * s4, qoff + 128 * (s4 + 1))
                        rra = prr.tile([64, 128], f32, tag="rra")
                        rrb = prr.tile([64, 128], f32, tag="rrb")
                        nc.vector.reciprocal(rra, oa[64:128, fs])
                        nc.vector.tensor_mul(oT_sb[0:64, qs4], oa[0:64, fs], rra)
                        nc.vector.reciprocal(rrb, ob[0:64, fs])
                        nc.vector.tensor_mul(oT_sb[64:128, qs4], ob[64:128, fs],
                                             rrb)
                        fine_tail[s4](tail=True)

            # Software pipeline over steps (c, pr): projections one step
            # ahead; output projections for chunk c trail after step (c, 1),
            # threaded through step (c+1, 1) (keeps ACT exp the pacer on the
            # lighter even steps and fills the mid-less last step).
            steps = [(c, pr) for c in range(4) for pr in range(2)]
            for s, (c, pr) in enumerate(steps):
                if s == 0:
                    for f in proj_pieces(0, 0):
                        f()
                mids = []
                if s + 1 < len(steps):
                    c2, pr2 = steps[s + 1]
                    mids = proj_pieces(pr2, c2)
                if pr == 1 and c >= 1:
                    opieces = [lambda s4=s4, cc=c - 1: oproj_piece(cc, s4)
                               for s4 in range(4)]
                    merged = []
                    while mids or opieces:
                        if mids:
                            merged.append(mids.pop(0))
                        if opieces:
                            merged.append(opieces.pop(0))
                    mids = merged
                fine = None
                if s == len(steps) - 1:
                    fine = [lambda tail=False, s4=s4: oproj_piece(3, s4, tail)
                            for s4 in range(4)]
                attn_chunk(pr, c, mids, fine_tail=fine)

    nc.compile()
    _RT.update(
        nc=nc, run_bass_kernel_spmd=run_bass_kernel_spmd, mybir=mybir,
    )
    return _RT


def _host_inputs(q_weight, k_weight, v_weight, o_weight, in_features):
    """Build the per-core input maps (host-side sharding + layout prep)."""
    import ml_dtypes
    bf = ml_dtypes.bfloat16
    x = np.ascontiguousarray(np.asarray(in_features, dtype=np.float32))
    qw = np.asarray(q_weight, dtype=np.float32)
    kw = np.asarray(k_weight, dtype=np.float32)
    vw = np.asarray(v_weight, dtype=np.float32)
    ow = np.asarray(o_weight, dtype=np.float32)

    perm64 = np.concatenate([np.arange(0, 64, 2), np.arange(1, 64, 2)])

    half = D_HEAD // 2
    inv_freq = THETA ** (-(np.arange(half, dtype=np.float64) * 2.0 / D_HEAD))
    pos = np.arange(S, dtype=np.float64)
    ang = inv_freq[:, None] * pos[None, :]        # [32, S]
    angf = np.tile(ang, (4, 1))                   # [128, S], row p -> i = p % 32
    trig = np.ascontiguousarray(np.stack(
        [np.cos(angf), np.sin(angf)], axis=1).astype(bf))

    spermT = np.zeros((128, 128), dtype=np.float32)
    for h in range(2):
        for i in range(32):
            spermT[h * 64 + 32 + i, h * 64 + i] = -1.0
            spermT[h * 64 + i, h * 64 + 32 + i] = 1.0
    spermT = spermT.astype(bf)

    kq = np.arange(128)
    mask128 = (np.arange(128)[None, :] >= kq[:, None]).astype(bf)

    f8 = ml_dtypes.float8_e4m3fn

    def xcomp(xb):
        """x as an fp8 (value, residual) pair in [128, 2, 8, S] layout."""
        xt = np.ascontiguousarray(xb.reshape(S, D_MODEL).T)  # [1024, S]
        x8 = xt.astype(f8)
        r8 = (xt - x8.astype(np.float32)).astype(f8)
        comp = np.stack([x8.reshape(8, 128, S), r8.reshape(8, 128, S)], axis=0)
        return np.ascontiguousarray(comp.transpose(2, 0, 1, 3))  # [128,2,8,S]

    def wcomp(wl):
        """256*w as fp8 w8 (duplicated DoubleRow planes) + residual pairs."""
        w256 = 256.0 * wl  # [1024, 256]
        w8 = w256.astype(f8)
        wr = (w256 - w8.astype(np.float32)).astype(f8)
        w8p = w8.reshape(8, 128, 256).transpose(1, 0, 2)      # [128, 8, 256]
        wx = np.ascontiguousarray(np.stack([w8p, w8p], axis=2))  # [128,8,2,256]
        wrp = wr.reshape(4, 2, 128, 256).transpose(2, 0, 1, 3)
        return wx, np.ascontiguousarray(wrp)                   # [128,4,2,256]

    xcb = [xcomp(x[b]) for b in range(B)]

    in_maps = []
    for core in range(N_CORES):
        b, g = divmod(core, 4)
        rows = slice(256 * g, 256 * (g + 1))

        def permqk(w):
            wc = w[rows]
            blocks = [wc[64 * h:64 * (h + 1)][perm64] for h in range(4)]
            return np.ascontiguousarray(np.concatenate(blocks).T)

        wqx, wqr = wcomp(permqk(qw))
        wkx, wkr = wcomp(permqk(kw))
        wvx, wvr = wcomp(np.ascontiguousarray(vw[rows].T))

        in_maps.append(dict(
            xc=xcb[b],
            wqx=wqx, wqr=wqr, wkx=wkx, wkr=wkr, wvx=wvx, wvr=wvr,
            wo=np.ascontiguousarray(ow[:, rows].T).astype(bf),
            trig=trig, sperm=spermT, mask128=mask128,
        ))
    return in_maps


def kernel(q_weight, k_weight, v_weight, o_weight, in_features):
    rt = _build()
    in_maps = _host_inputs(q_weight, k_weight, v_weight, o_weight, in_features)
    res = rt["run_bass_kernel_spmd"](
        rt["nc"], in_maps, core_ids=list(range(N_CORES)),
    )
    y = np.zeros((B, S, D_MODEL), dtype=np.float32)
    for core in range(N_CORES):
        y[core // 4] += np.asarray(res.results[core]["y"], dtype=np.float32)
    return y
